# revision 28
# baseline (speedup 1.0000x reference)
"""DiffAttn kernel for 8 Trainium2 NeuronCores.

Problem: out = softmax(Q1 K1^T / sqrt(d)) V - lam * softmax(Q2 K2^T / sqrt(d)) V
with Q = X W_q, K = X W_k, V = X W_v;  X [2, 4096, 1024], W [1024, 128], d = 64.

Sharding: 8 cores = (batch b, query-chunk qc) with b = core // 4, qc = core % 4.
Each core receives its batch's X rolled so that its 1024 query rows come first
(attention is permutation-invariant over keys).  Each core computes the full
K/V projection for its batch, Q for its query chunk, and two-branch flash
attention without max-subtraction (scores ~N(0,1); exp(s*0.125 - 3) stays in
fp8e4 range), normalizing at the end: out = U1/r1 - lam*U2/r2.

Pipeline (one TileContext):
  scope 1 (per seq tile st): DMA X rows -> PE transpose (f32r) -> K^T/V/Q
  projections (bf16) -> copies to fp8; interleave attention score matmuls
  (fp8 DoubleRow over d split 2x32) + exp on the Activation engine, retaining
  exp tiles in SBUF for all 64 (kt) x 2 (qt) tiles.
  scope 2: PV matmuls (fp8 DoubleRow over key-tile pairs) + row-sums via
  stationary-E 1-column matmuls, then the normalize/combine epilogue.
"""

import sys

if '/opt/trn_rl_repo' not in sys.path:
    sys.path.insert(0, '/opt/trn_rl_repo')

import numpy as np

B, S, DIN, D = 2, 4096, 1024, 64
TD = 2 * D            # 128: both branches' head dims
NQ = S // 4           # 1024 query rows per core
ST = 512              # seq tile
NST = S // ST         # 8
QT = 512              # query tile
NQT = NQ // QT        # 2
KT = 128              # key tile
NKT = S // KT         # 32
NDC = DIN // 128      # 8 contraction chunks

SCORES_FP8 = False     # fp8 DoubleRow score matmuls (else bf16)


def build_nc(scores_fp8=SCORES_FP8):
    import concourse.bacc as bacc
    import concourse.mybir as mybir
    from concourse.tile import TileContext
    from concourse.masks import make_identity

    F32 = mybir.dt.float32
    F32R = mybir.dt.float32r
    BF16 = mybir.dt.bfloat16
    F8 = mybir.dt.float8e4
    AF = mybir.ActivationFunctionType
    DR = mybir.MatmulPerfMode.DoubleRow

    nc = bacc.Bacc("TRN2", target_bir_lowering=False)
    X_t = nc.dram_tensor("X", [S, DIN], F32, kind="ExternalInput")
    Wq_t = nc.dram_tensor("Wq", [DIN, TD], F32, kind="ExternalInput")
    Wk_t = nc.dram_tensor("Wk", [DIN, TD], F32, kind="ExternalInput")
    Wv_t = nc.dram_tensor("Wv", [DIN, TD], F32, kind="ExternalInput")
    lam_t = nc.dram_tensor("lam", [1, 1], F32, kind="ExternalInput")
    out_t = nc.dram_tensor("out", [NQ, TD], F32, kind="ExternalOutput")

    with TileContext(nc) as tc:
        with tc.tile_pool(name="consts", bufs=1) as consts, \
             tc.tile_pool(name="pers", bufs=1) as pers, \
             tc.tile_pool(name="wst", bufs=2) as wst:
            ident32 = consts.tile([128, 128], F32, tag="ident32")
            make_identity(nc, ident32)
            ones8 = consts.tile([128, 1], BF16, tag="ones8")
            nc.vector.memset(ones8, 1.0)
            nbias = consts.tile([128, 1], F32, tag="nbias")
            nc.vector.memset(nbias, -2.35)
            lam_sb = consts.tile([128, 1], F32, tag="lam")
            nc.gpsimd.dma_start(
                out=lam_sb, in_=lam_t.ap().partition_broadcast(128))

            # K^T/Q^T storage
            if scores_fp8:
                k8n = pers.tile([128, S], F8, tag="k8n")
                q8n = pers.tile([128, NQ], F8, tag="q8n")
                k8d = pers.tile([64, 2, S], F8, tag="k8d")
                q8d = pers.tile([64, 2, NQ], F8, tag="q8d")
            else:
                kTb = pers.tile([128, S], BF16, tag="kTb")
                qTb = pers.tile([128, NQ], BF16, tag="qTb")
            v_sb = pers.tile([128, NKT, TD], BF16, tag="v")
            # retained exp tiles: [128, kt, branch, q]
            e_all = []
            for qt in range(NQT):
                e_qt = pers.tile([128, 2, NKT, QT], BF16, tag=f"e{qt}")
                e_all.append(e_qt)

            # ---------- scope 1: projections + scores/exp ----------
            with tc.tile_pool(name="xn", bufs=5) as xn_pool, \
                 tc.tile_pool(name="xt", bufs=2) as xt_pool, \
                 tc.tile_pool(name="tp", bufs=2, space="PSUM") as tp_ps, \
                 tc.tile_pool(name="pj", bufs=2, space="PSUM") as pj_ps, \
                 tc.tile_pool(name="s", bufs=2, space="PSUM") as s_ps:

                def emit_kt(qt, kt):
                    """scores + exp for key tile kt against q tile qt."""
                    q0 = qt * QT
                    k0 = kt * KT
                    s12 = s_ps.tile([128, 2, QT], F32, tag="s")
                    if scores_fp8:
                        for br in range(2):
                            nc.tensor.matmul(
                                s12[:, br, :],
                                k8d[32 * br:32 * br + 32, :, k0:k0 + KT],
                                q8d[32 * br:32 * br + 32, :, q0:q0 + QT],
                                start=True, stop=True, perf_mode=DR)
                    else:
                        for br in range(2):
                            nc.tensor.matmul(
                                s12[:, br, :],
                                kTb[64 * br:64 * br + 64, k0:k0 + KT],
                                qTb[64 * br:64 * br + 64, q0:q0 + QT],
                                start=True, stop=True)
                    nc.scalar.activation(
                        out=e_all[qt][:, :, kt, :], in_=s12,
                        func=AF.Exp, scale=0.125, bias=nbias)

                def load_xns(st):
                    s0 = st * ST
                    lst = []
                    for sp in range(4):
                        xn = xn_pool.tile([128, DIN], F32, tag="xn")
                        nc.sync.dma_start(
                            out=xn, in_=X_t.ap()[s0 + sp * 128:
                                                 s0 + (sp + 1) * 128, :])
                        lst.append(xn)
                    return lst

                # prefetch X for the first two seq tiles ahead of the weights
                xns_bufs = [load_xns(0), load_xns(1)]

                # weights -> bf16 [128 part = din sub-chunk, NDC, TD]
                w_sb = {}
                for name, t in (("wq", Wq_t), ("wk", Wk_t), ("wv", Wv_t)):
                    stg = wst.tile([128, NDC, TD], F32, tag="wstg")
                    nc.sync.dma_start(
                        out=stg,
                        in_=t.ap().rearrange("(c p) n -> p c n", p=128))
                    w = pers.tile([128, NDC, TD], BF16, tag=name)
                    nc.vector.tensor_copy(w, stg)
                    w_sb[name] = w

                for st in range(NST):
                    s0 = st * ST
                    xns = xns_bufs.pop(0)
                    if st + 2 < NST:
                        xns_bufs.append(load_xns(st + 2))
                    # transpose to X^T tile [128 (din chunk), NDC, ST] bf16
                    xt = xt_pool.tile([128, NDC, ST], BF16, tag="xt")
                    for dc in range(NDC):
                        tp = tp_ps.tile([128, ST], F32, tag="tp")
                        for sp in range(4):
                            nc.tensor.matmul(
                                tp[:, sp * 128:(sp + 1) * 128],
                                xns[sp][:, dc * 128:(dc + 1) * 128],
                                ident32, is_transpose=True,
                                start=(sp == 0), stop=(sp == 3))
                        nc.vector.tensor_copy(xt[:, dc, :], tp)

                    # K^T slice [TD, ST]
                    kps = pj_ps.tile([128, ST], F32, tag="pj")
                    for dc in range(NDC):
                        nc.tensor.matmul(
                            kps, w_sb["wk"][:, dc, :], xt[:, dc, :],
                            start=(dc == 0), stop=(dc == NDC - 1))
                    if scores_fp8:
                        nc.vector.tensor_copy(k8n[:, s0:s0 + ST], kps)
                        for b in range(2):
                            for j in range(2):
                                nc.sync.dma_start(
                                    out=k8d[32 * b:32 * b + 32, j,
                                            s0:s0 + ST],
                                    in_=k8n[64 * b + 32 * j:
                                            64 * b + 32 * j + 32,
                                            s0:s0 + ST])
                    else:
                        nc.vector.tensor_copy(kTb[:, s0:s0 + ST], kps)

                    # Q^T slice (first NQ rows only)
                    if st < NQ // ST:
                        qps = pj_ps.tile([128, ST], F32, tag="pj")
                        for dc in range(NDC):
                            nc.tensor.matmul(
                                qps, w_sb["wq"][:, dc, :], xt[:, dc, :],
                                start=(dc == 0), stop=(dc == NDC - 1))
                        if scores_fp8:
                            nc.vector.tensor_copy(q8n[:, s0:s0 + ST], qps)
                            for b in range(2):
                                for j in range(2):
                                    nc.sync.dma_start(
                                        out=q8d[32 * b:32 * b + 32, j,
                                                s0:s0 + ST],
                                        in_=q8n[64 * b + 32 * j:
                                                64 * b + 32 * j + 32,
                                                s0:s0 + ST])
                        else:
                            nc.vector.tensor_copy(qTb[:, s0:s0 + ST], qps)

                    # V natural [s, TD] for this seq tile, interleaved with
                    # this st's qt0 scores and last st's qt1 scores
                    for sp in range(4):
                        if sp == 0:
                            emit_kt(0, 4 * st + 0)
                        elif sp == 2:
                            emit_kt(0, 4 * st + 1)
                        vps = pj_ps.tile([128, ST], F32, tag="pj")
                        for dc in range(NDC):
                            nc.tensor.matmul(
                                vps[:, 0:TD],
                                xt[:, dc, sp * 128:(sp + 1) * 128],
                                w_sb["wv"][:, dc, :],
                                start=(dc == 0), stop=(dc == NDC - 1))
                        nc.vector.tensor_copy(
                            v_sb[:, st * 4 + sp, :], vps[:, 0:TD])
                    emit_kt(0, 4 * st + 2)
                    emit_kt(0, 4 * st + 3)
                    if st >= 1:
                        for t in range(4):
                            emit_kt(1, 4 * (st - 1) + t)

                # leftover group
                for t in range(4):
                    emit_kt(1, 4 * (NST - 1) + t)

            # ---------- scope 2: PV + row sums + epilogue ----------
            with tc.tile_pool(name="o", bufs=1, space="PSUM") as o_ps, \
                 tc.tile_pool(name="r", bufs=2, space="PSUM") as r_ps, \
                 tc.tile_pool(name="t", bufs=1, space="PSUM") as t_ps, \
                 tc.tile_pool(name="usb", bufs=1) as usb, \
                 tc.tile_pool(name="osb", bufs=2) as osb:
                import concourse.mybir as _mb
                NP = NKT // 2
                for qt in range(NQT):
                    q0 = qt * QT
                    e = e_all[qt]
                    o1 = o_ps.tile([128, QT], F32, tag="o1")
                    o2 = o_ps.tile([128, QT], F32, tag="o2")
                    for kt in range(NKT):
                        first, last = (kt == 0), (kt == NKT - 1)
                        vv = v_sb[:, kt, :]
                        nc.tensor.matmul(o1, vv, e[:, 0, kt, :],
                                         start=first, stop=last)
                        nc.tensor.matmul(o2, vv, e[:, 1, kt, :],
                                         start=first, stop=last)

                    # row sums: stationary-E 1-column DoubleRow matmuls,
                    # one accumulation group at a time per psum bank
                    rinv = {}
                    for br in range(2):
                        for qc in range(4):
                            # full-bank tile: start=True zeroes a whole 2KB
                            # psum bank, so accumulators must not share banks
                            rs = r_ps.tile([128, 512], F32, tag="rs")
                            for kt in range(NKT):
                                nc.tensor.matmul(
                                    rs[:, 0:1],
                                    e[:, br, kt,
                                      qc * 128:(qc + 1) * 128],
                                    ones8,
                                    start=(kt == 0), stop=(kt == NKT - 1))
                            ri = usb.tile([128, 1], F32, tag=f"ri{br}{qc}")
                            nc.vector.reciprocal(ri, rs[:, 0:1])
                            rinv[(br, qc)] = ri

                    # epilogue: normalize and write out
                    u1 = usb.tile([128, QT], F32, tag="u1")
                    u2 = usb.tile([128, QT], F32, tag="u2")
                    nc.vector.tensor_copy(u1, o1)
                    nc.vector.tensor_copy(u2, o2)
                    for qc in range(4):
                        c0 = qc * 128
                        rc2 = osb.tile([128, 1], F32, tag="rc2")
                        nc.vector.tensor_mul(rc2, rinv[(1, qc)], lam_sb)
                        ut1 = t_ps.tile([128, 512], F32, tag="ut1")
                        ut2 = t_ps.tile([128, 512], F32, tag="ut2")
                        nc.tensor.matmul(
                            ut1[:, 0:TD], u1[:, c0:c0 + 128], ident32,
                            is_transpose=True, start=True, stop=True)
                        nc.tensor.matmul(
                            ut2[:, 0:TD], u2[:, c0:c0 + 128], ident32,
                            is_transpose=True, start=True, stop=True)
                        tmp = osb.tile([128, TD], F32, tag="tmp")
                        nc.vector.tensor_scalar_mul(
                            tmp, ut2[:, 0:TD], rc2)
                        ob = osb.tile([128, TD], F32, tag="ob")
                        nc.vector.scalar_tensor_tensor(
                            out=ob, in0=ut1[:, 0:TD],
                            scalar=rinv[(0, qc)], in1=tmp,
                            op0=_mb.AluOpType.mult,
                            op1=_mb.AluOpType.subtract)
                        nc.sync.dma_start(
                            out=out_t.ap()[q0 + c0:q0 + c0 + 128, :], in_=ob)

    nc.compile()
    return nc


_NC_CACHE = None


def kernel(X, W_q, W_k, W_v, lam):
    global _NC_CACHE
    from concourse.bass_utils import run_bass_kernel_spmd

    X = np.asarray(X, dtype=np.float32)
    W_q = np.asarray(W_q, dtype=np.float32)
    W_k = np.asarray(W_k, dtype=np.float32)
    W_v = np.asarray(W_v, dtype=np.float32)
    lam_arr = np.asarray(lam, dtype=np.float32).reshape(1, 1)

    if _NC_CACHE is None:
        _NC_CACHE = build_nc()
    nc = _NC_CACHE

    in_maps = []
    for c in range(8):
        b, qc = divmod(c, 4)
        qs = qc * NQ
        Xc = np.ascontiguousarray(
            np.concatenate([X[b, qs:], X[b, :qs]], axis=0))
        in_maps.append({"X": Xc, "Wq": W_q, "Wk": W_k, "Wv": W_v,
                        "lam": lam_arr})

    res = run_bass_kernel_spmd(nc, in_maps, core_ids=list(range(8)))

    out = np.empty((B, S, TD), dtype=np.float32)
    for c in range(8):
        b, qc = divmod(c, 4)
        qs = qc * NQ
        out[b, qs:qs + NQ] = res.results[c]["out"]
    return out


# revision 34
# speedup vs baseline: 1.0110x; 1.0110x over previous
"""DiffAttn kernel for 8 Trainium2 NeuronCores.

Problem: out = softmax(Q1 K1^T / sqrt(d)) V - lam * softmax(Q2 K2^T / sqrt(d)) V
with Q = X W_q, K = X W_k, V = X W_v;  X [2, 4096, 1024], W [1024, 128], d = 64.

Sharding: 8 cores = (batch b, query-chunk qc) with b = core // 4, qc = core % 4.
Each core receives its batch's X rolled so that its 1024 query rows come first
(attention is permutation-invariant over keys).  Each core computes the full
K/V projection for its batch, Q for its query chunk, and two-branch flash
attention without max-subtraction (scores are ~N(0,1); exp(s*0.125 - 2.35)
is safe in bf16), normalizing at the end: out = U1/r1 - lam*U2/r2.

Pipeline (one TileContext):
  scope 1 (per seq tile st): DMA X rows -> PE transpose -> K^T/V/Q
  projections (bf16, V computed in natural layout directly); interleave
  attention score matmuls (bf16) + exp on the Activation engine, retaining
  exp tiles (bf16) in SBUF for all 64 (kt) x 2 (qt) tiles.
  scope 2: PV matmuls + row-sums via stationary-E 1-column matmuls (each
  accumulator in its own 2KB psum bank), then the normalize epilogue.
"""

import sys

if '/opt/trn_rl_repo' not in sys.path:
    sys.path.insert(0, '/opt/trn_rl_repo')

import numpy as np

B, S, DIN, D = 2, 4096, 1024, 64
TD = 2 * D            # 128: both branches' head dims
NQ = S // 4           # 1024 query rows per core
ST = 512              # seq tile
NST = S // ST         # 8
QT = 512              # query tile
NQT = NQ // QT        # 2
KT = 128              # key tile
NKT = S // KT         # 32
NDC = DIN // 128      # 8 contraction chunks

SCORES_FP8 = False     # fp8 DoubleRow score matmuls (else bf16)


def build_nc(scores_fp8=SCORES_FP8):
    import concourse.bacc as bacc
    import concourse.mybir as mybir
    from concourse.tile import TileContext
    from concourse.masks import make_identity

    F32 = mybir.dt.float32
    F32R = mybir.dt.float32r
    BF16 = mybir.dt.bfloat16
    F8 = mybir.dt.float8e4
    AF = mybir.ActivationFunctionType
    DR = mybir.MatmulPerfMode.DoubleRow

    nc = bacc.Bacc("TRN2", target_bir_lowering=False)
    X_t = nc.dram_tensor("X", [S, DIN], F32, kind="ExternalInput")
    Wq_t = nc.dram_tensor("Wq", [DIN, TD], F32, kind="ExternalInput")
    Wk_t = nc.dram_tensor("Wk", [DIN, TD], F32, kind="ExternalInput")
    Wv_t = nc.dram_tensor("Wv", [DIN, TD], F32, kind="ExternalInput")
    lam_t = nc.dram_tensor("lam", [1, 1], F32, kind="ExternalInput")
    out_t = nc.dram_tensor("out", [NQ, TD], F32, kind="ExternalOutput")

    with TileContext(nc) as tc:
        with tc.tile_pool(name="consts", bufs=1) as consts, \
             tc.tile_pool(name="pers", bufs=1) as pers, \
             tc.tile_pool(name="wst", bufs=2) as wst:
            ident32 = consts.tile([128, 128], F32, tag="ident32")
            make_identity(nc, ident32)
            ident_r = consts.tile([128, 128], F32R, tag="ident_r")
            nc.scalar.copy(out=ident_r, in_=ident32)
            ones8 = consts.tile([128, 1], BF16, tag="ones8")
            nc.vector.memset(ones8, 1.0)
            nbias = consts.tile([128, 1], F32, tag="nbias")
            nc.vector.memset(nbias, -2.35)
            lam_sb = consts.tile([128, 1], F32, tag="lam")
            nc.gpsimd.dma_start(
                out=lam_sb, in_=lam_t.ap().partition_broadcast(128))

            # K^T/Q^T storage
            if scores_fp8:
                k8n = pers.tile([128, S], F8, tag="k8n")
                q8n = pers.tile([128, NQ], F8, tag="q8n")
                k8d = pers.tile([64, 2, S], F8, tag="k8d")
                q8d = pers.tile([64, 2, NQ], F8, tag="q8d")
            else:
                kTb = pers.tile([128, S], BF16, tag="kTb")
                qTb = pers.tile([128, NQ], BF16, tag="qTb")
            v_sb = pers.tile([128, NKT, TD], BF16, tag="v")
            # retained exp tiles: [128, kt, branch, q]
            e_all = []
            for qt in range(NQT):
                e_qt = pers.tile([128, 2, NKT, QT], BF16, tag=f"e{qt}")
                e_all.append(e_qt)

            # ---------- scope 1: projections + scores/exp ----------
            with tc.tile_pool(name="xn", bufs=5) as xn_pool, \
                 tc.tile_pool(name="xt", bufs=2) as xt_pool, \
                 tc.tile_pool(name="tp", bufs=2, space="PSUM") as tp_ps, \
                 tc.tile_pool(name="pj", bufs=2, space="PSUM") as pj_ps, \
                 tc.tile_pool(name="s", bufs=2, space="PSUM") as s_ps, \
                 tc.tile_pool(name="usb", bufs=1) as usb, \
                 tc.tile_pool(name="osb", bufs=2) as osb:

                def emit_kt(qt, kt):
                    """scores + exp for key tile kt against q tile qt."""
                    q0 = qt * QT
                    k0 = kt * KT
                    s12 = s_ps.tile([128, 2, QT], F32, tag="s")
                    if scores_fp8:
                        for br in range(2):
                            nc.tensor.matmul(
                                s12[:, br, :],
                                k8d[32 * br:32 * br + 32, :, k0:k0 + KT],
                                q8d[32 * br:32 * br + 32, :, q0:q0 + QT],
                                start=True, stop=True, perf_mode=DR)
                    else:
                        for br in range(2):
                            nc.tensor.matmul(
                                s12[:, br, :],
                                kTb[64 * br:64 * br + 64, k0:k0 + KT],
                                qTb[64 * br:64 * br + 64, q0:q0 + QT],
                                start=True, stop=True)
                    nc.scalar.activation(
                        out=e_all[qt][:, :, kt, :], in_=s12,
                        func=AF.Exp, scale=0.125, bias=nbias)

                def load_xns(st):
                    s0 = st * ST
                    lst = []
                    for sp in range(4):
                        xn = xn_pool.tile([128, DIN], F32R, tag="xn")
                        nc.sync.dma_start(
                            out=xn, in_=X_t.ap()[s0 + sp * 128:
                                                 s0 + (sp + 1) * 128, :]
                            .bitcast(F32R))
                        lst.append(xn)
                    return lst

                # prefetch X for the first two seq tiles ahead of the weights
                xns_bufs = [load_xns(0), load_xns(1)]

                # weights -> bf16 [128 part = din sub-chunk, NDC, TD]
                w_sb = {}
                for name, t in (("wq", Wq_t), ("wk", Wk_t), ("wv", Wv_t)):
                    stg = wst.tile([128, NDC, TD], F32, tag="wstg")
                    nc.sync.dma_start(
                        out=stg,
                        in_=t.ap().rearrange("(c p) n -> p c n", p=128))
                    w = pers.tile([128, NDC, TD], BF16, tag=name)
                    nc.scalar.copy(out=w, in_=stg)
                    w_sb[name] = w

                for st in range(NST):
                    s0 = st * ST
                    xns = xns_bufs.pop(0)
                    if st + 2 < NST:
                        xns_bufs.append(load_xns(st + 2))
                    # transpose to X^T tile [128 (din chunk), NDC, ST] bf16
                    xt = xt_pool.tile([128, NDC, ST], BF16, tag="xt")
                    for dc in range(NDC):
                        tp = tp_ps.tile([128, ST], F32R, tag="tp")
                        for sp in range(4):
                            nc.tensor.matmul(
                                tp[:, sp * 128:(sp + 1) * 128],
                                xns[sp][:, dc * 128:(dc + 1) * 128],
                                ident_r, is_transpose=True,
                                start=(sp == 0), stop=(sp == 3))
                        if st == 0:
                            nc.scalar.copy(out=xt[:, dc, :],
                                           in_=tp[:, :].bitcast(F32))
                        else:
                            nc.vector.tensor_copy(
                                xt[:, dc, :], tp[:, :].bitcast(F32))

                    # K^T slice [TD, ST]
                    kps = pj_ps.tile([128, ST], F32, tag="pj")
                    for dc in range(NDC):
                        nc.tensor.matmul(
                            kps, w_sb["wk"][:, dc, :], xt[:, dc, :],
                            start=(dc == 0), stop=(dc == NDC - 1))
                    if scores_fp8:
                        nc.vector.tensor_copy(k8n[:, s0:s0 + ST], kps)
                        for b in range(2):
                            for j in range(2):
                                nc.sync.dma_start(
                                    out=k8d[32 * b:32 * b + 32, j,
                                            s0:s0 + ST],
                                    in_=k8n[64 * b + 32 * j:
                                            64 * b + 32 * j + 32,
                                            s0:s0 + ST])
                    else:
                        nc.vector.tensor_copy(kTb[:, s0:s0 + ST], kps)

                    # Q^T slice (first NQ rows only)
                    if st < NQ // ST:
                        qps = pj_ps.tile([128, ST], F32, tag="pj")
                        for dc in range(NDC):
                            nc.tensor.matmul(
                                qps, w_sb["wq"][:, dc, :], xt[:, dc, :],
                                start=(dc == 0), stop=(dc == NDC - 1))
                        if scores_fp8:
                            nc.vector.tensor_copy(q8n[:, s0:s0 + ST], qps)
                            for b in range(2):
                                for j in range(2):
                                    nc.sync.dma_start(
                                        out=q8d[32 * b:32 * b + 32, j,
                                                s0:s0 + ST],
                                        in_=q8n[64 * b + 32 * j:
                                                64 * b + 32 * j + 32,
                                                s0:s0 + ST])
                        else:
                            nc.vector.tensor_copy(qTb[:, s0:s0 + ST], qps)

                    # V natural [s, TD] for this seq tile, interleaved with
                    # this st's qt0 scores and last st's qt1 scores
                    for sp in range(4):
                        if st == 0:
                            if sp == 0:
                                emit_kt(0, 0)
                            elif sp == 2:
                                emit_kt(0, 1)
                        elif sp < 2:
                            emit_kt(0, 4 * st + sp)
                        else:
                            emit_kt(1, 4 * (st - 1) + (sp - 2))
                        vps = pj_ps.tile([128, ST], F32, tag="pj")
                        for dc in range(NDC):
                            nc.tensor.matmul(
                                vps[:, 0:TD],
                                xt[:, dc, sp * 128:(sp + 1) * 128],
                                w_sb["wv"][:, dc, :],
                                start=(dc == 0), stop=(dc == NDC - 1))
                        nc.vector.tensor_copy(
                            v_sb[:, st * 4 + sp, :], vps[:, 0:TD])
                    if st == 0:
                        emit_kt(0, 2)
                        emit_kt(0, 3)
                    else:
                        emit_kt(0, 4 * st + 2)
                        emit_kt(1, 4 * (st - 1) + 2)
                        emit_kt(0, 4 * st + 3)
                        emit_kt(1, 4 * (st - 1) + 3)

                # tail: last qt1 scores interleaved with qt0 PV
                import concourse.mybir as _mb

                def pv_mms(o1, o2, e, k0, k1):
                    for kt in range(k0, k1):
                        first, last = (kt == 0), (kt == NKT - 1)
                        vv = v_sb[:, kt, :]
                        nc.tensor.matmul(o1, vv, e[:, 0, kt, :],
                                         start=first, stop=last)
                        nc.tensor.matmul(o2, vv, e[:, 1, kt, :],
                                         start=first, stop=last)

                def rowsums(e):
                    rinv = {}
                    for br in range(2):
                        for qc in range(4):
                            rs = tp_ps.tile([128, ST], F32R, tag="tp")
                            rsf = rs[:, :].bitcast(F32)
                            for kt in range(NKT):
                                nc.tensor.matmul(
                                    rsf[:, 0:1],
                                    e[:, br, kt, qc * 128:(qc + 1) * 128],
                                    ones8,
                                    start=(kt == 0), stop=(kt == NKT - 1))
                            ri = usb.tile([128, 1], F32, tag=f"ri{br}{qc}")
                            nc.vector.reciprocal(ri, rsf[:, 0:1])
                            rinv[(br, qc)] = ri
                    return rinv

                def epilogue(qt, o1, o2, rinv):
                    q0 = qt * QT
                    u1 = usb.tile([128, QT], F32, tag="u1")
                    u2 = usb.tile([128, QT], F32, tag="u2")
                    nc.vector.tensor_copy(u1, o1)
                    nc.vector.tensor_copy(u2, o2)
                    for qc in range(4):
                        c0 = qc * 128
                        rc2 = osb.tile([128, 1], F32, tag="rc2")
                        nc.vector.tensor_mul(rc2, rinv[(1, qc)], lam_sb)
                        uts = s_ps.tile([128, 2, QT], F32, tag="s")
                        nc.tensor.matmul(
                            uts[:, 0, 0:TD], u1[:, c0:c0 + 128], ident32,
                            is_transpose=True, start=True, stop=True)
                        nc.tensor.matmul(
                            uts[:, 1, 0:TD], u2[:, c0:c0 + 128], ident32,
                            is_transpose=True, start=True, stop=True)
                        tmp = osb.tile([128, TD], F32, tag="tmp")
                        nc.vector.tensor_scalar_mul(
                            tmp, uts[:, 1, 0:TD], rc2)
                        ob = osb.tile([128, TD], F32, tag="ob")
                        nc.vector.scalar_tensor_tensor(
                            out=ob, in0=uts[:, 0, 0:TD],
                            scalar=rinv[(0, qc)], in1=tmp,
                            op0=_mb.AluOpType.mult,
                            op1=_mb.AluOpType.subtract)
                        nc.sync.dma_start(
                            out=out_t.ap()[q0 + c0:q0 + c0 + 128, :], in_=ob)

                o1_0 = pj_ps.tile([128, ST], F32, tag="pj")
                o2_0 = pj_ps.tile([128, ST], F32, tag="pj")
                for t in range(4):
                    emit_kt(1, 4 * (NST - 1) + t)
                    pv_mms(o1_0, o2_0, e_all[0], 8 * t, 8 * (t + 1))
                rinv0 = rowsums(e_all[0])
                epilogue(0, o1_0, o2_0, rinv0)

                o1_1 = pj_ps.tile([128, ST], F32, tag="pj")
                o2_1 = pj_ps.tile([128, ST], F32, tag="pj")
                pv_mms(o1_1, o2_1, e_all[1], 0, NKT)
                rinv1 = rowsums(e_all[1])
                epilogue(1, o1_1, o2_1, rinv1)

    nc.compile()
    return nc


_NC_CACHE = None


def kernel(X, W_q, W_k, W_v, lam):
    global _NC_CACHE
    from concourse.bass_utils import run_bass_kernel_spmd

    X = np.asarray(X, dtype=np.float32)
    W_q = np.asarray(W_q, dtype=np.float32)
    W_k = np.asarray(W_k, dtype=np.float32)
    W_v = np.asarray(W_v, dtype=np.float32)
    lam_arr = np.asarray(lam, dtype=np.float32).reshape(1, 1)

    if _NC_CACHE is None:
        _NC_CACHE = build_nc()
    nc = _NC_CACHE

    in_maps = []
    for c in range(8):
        b, qc = divmod(c, 4)
        qs = qc * NQ
        Xc = np.ascontiguousarray(
            np.concatenate([X[b, qs:], X[b, :qs]], axis=0))
        in_maps.append({"X": Xc, "Wq": W_q, "Wk": W_k, "Wv": W_v,
                        "lam": lam_arr})

    res = run_bass_kernel_spmd(nc, in_maps, core_ids=list(range(8)))

    out = np.empty((B, S, TD), dtype=np.float32)
    for c in range(8):
        b, qc = divmod(c, 4)
        qs = qc * NQ
        out[b, qs:qs + NQ] = res.results[c]["out"]
    return out


# revision 40
# speedup vs baseline: 1.0425x; 1.0312x over previous
"""DiffAttn kernel for 8 Trainium2 NeuronCores.

Problem: out = softmax(Q1 K1^T / sqrt(d)) V - lam * softmax(Q2 K2^T / sqrt(d)) V
with Q = X W_q, K = X W_k, V = X W_v;  X [2, 4096, 1024], W [1024, 128], d = 64.

Sharding: 8 cores = (batch b, query-chunk qc) with b = core // 4, qc = core % 4.
Each core receives its batch's X rolled so that its 1024 query rows come first
(attention is permutation-invariant over keys).  Each core computes the full
K/V projection for its batch, Q for its query chunk, and two-branch flash
attention without max-subtraction (scores are ~N(0,1); exp(s*0.125 - 2.35)
is safe in bf16), normalizing at the end: out = U1/r1 - lam*U2/r2.

Pipeline (one TileContext):
  scope 1 (per seq tile st): DMA X rows -> PE transpose -> K^T/V/Q
  projections (bf16, V computed in natural layout directly); interleave
  attention score matmuls (bf16) + exp on the Activation engine, retaining
  exp tiles (bf16) in SBUF for all 64 (kt) x 2 (qt) tiles.
  scope 2: PV matmuls + row-sums via stationary-E 1-column matmuls (each
  accumulator in its own 2KB psum bank), then the normalize epilogue.
"""

import sys

if '/opt/trn_rl_repo' not in sys.path:
    sys.path.insert(0, '/opt/trn_rl_repo')

import numpy as np

B, S, DIN, D = 2, 4096, 1024, 64
TD = 2 * D            # 128: both branches' head dims
NQ = S // 4           # 1024 query rows per core
ST = 512              # seq tile
NST = S // ST         # 8
QT = 512              # query tile
NQT = NQ // QT        # 2
KT = 128              # key tile
NKT = S // KT         # 32
NDC = DIN // 128      # 8 contraction chunks

SCORES_FP8 = False     # fp8 DoubleRow score matmuls (else bf16)


def build_nc(scores_fp8=SCORES_FP8):
    import concourse.bacc as bacc
    import concourse.mybir as mybir
    from concourse.tile import TileContext
    from concourse.masks import make_identity

    F32 = mybir.dt.float32
    F32R = mybir.dt.float32r
    BF16 = mybir.dt.bfloat16
    F8 = mybir.dt.float8e4
    AF = mybir.ActivationFunctionType
    DR = mybir.MatmulPerfMode.DoubleRow

    nc = bacc.Bacc("TRN2", target_bir_lowering=False)
    X_t = nc.dram_tensor("X", [S, DIN], F32, kind="ExternalInput")
    Wq_t = nc.dram_tensor("Wq", [DIN, TD], F32, kind="ExternalInput")
    Wk_t = nc.dram_tensor("Wk", [DIN, TD], F32, kind="ExternalInput")
    Wv_t = nc.dram_tensor("Wv", [DIN, TD], F32, kind="ExternalInput")
    lam_t = nc.dram_tensor("lam", [1, 1], F32, kind="ExternalInput")
    out_t = nc.dram_tensor("out", [NQ, TD], F32, kind="ExternalOutput")

    with TileContext(nc) as tc:
        with tc.tile_pool(name="consts", bufs=1) as consts, \
             tc.tile_pool(name="pers", bufs=1) as pers, \
             tc.tile_pool(name="wst", bufs=2) as wst:
            ident32 = consts.tile([128, 128], F32, tag="ident32")
            make_identity(nc, ident32)
            ident_r = consts.tile([128, 128], F32R, tag="ident_r")
            nc.scalar.copy(out=ident_r, in_=ident32)
            ones8 = consts.tile([128, 1], BF16, tag="ones8")
            nc.vector.memset(ones8, 1.0)
            nbias = consts.tile([128, 1], F32, tag="nbias")
            nc.vector.memset(nbias, -2.35)
            lam_sb = consts.tile([128, 1], F32, tag="lam")
            nc.gpsimd.dma_start(
                out=lam_sb, in_=lam_t.ap().partition_broadcast(128))

            # K^T/Q^T storage
            if scores_fp8:
                k8n = pers.tile([128, S], F8, tag="k8n")
                q8n = pers.tile([128, NQ], F8, tag="q8n")
                k8d = pers.tile([64, 2, S], F8, tag="k8d")
                q8d = pers.tile([64, 2, NQ], F8, tag="q8d")
            else:
                kTb = pers.tile([128, S], BF16, tag="kTb")
                qTb = pers.tile([128, NQ], BF16, tag="qTb")
            v_sb = pers.tile([128, NKT, TD], BF16, tag="v")
            # retained exp tiles: [128, kt, branch, q]
            e_all = []
            for qt in range(NQT):
                e_qt = pers.tile([128, 2, NKT, QT], BF16, tag=f"e{qt}")
                e_all.append(e_qt)

            # ---------- scope 1: projections + scores/exp ----------
            with tc.tile_pool(name="xn", bufs=6) as xn_pool, \
                 tc.tile_pool(name="xt", bufs=2) as xt_pool, \
                 tc.tile_pool(name="tp", bufs=2, space="PSUM") as tp_ps, \
                 tc.tile_pool(name="pj", bufs=2, space="PSUM") as pj_ps, \
                 tc.tile_pool(name="s", bufs=2, space="PSUM") as s_ps, \
                 tc.tile_pool(name="usb", bufs=1) as usb, \
                 tc.tile_pool(name="osb", bufs=2) as osb:

                def emit_kt(qt, kt):
                    """scores + exp for key tile kt against q tile qt."""
                    q0 = qt * QT
                    k0 = kt * KT
                    s12 = s_ps.tile([128, 2, QT], F32, tag="s")
                    if scores_fp8:
                        for br in range(2):
                            nc.tensor.matmul(
                                s12[:, br, :],
                                k8d[32 * br:32 * br + 32, :, k0:k0 + KT],
                                q8d[32 * br:32 * br + 32, :, q0:q0 + QT],
                                start=True, stop=True, perf_mode=DR)
                    else:
                        for br in range(2):
                            nc.tensor.matmul(
                                s12[:, br, :],
                                kTb[64 * br:64 * br + 64, k0:k0 + KT],
                                qTb[64 * br:64 * br + 64, q0:q0 + QT],
                                start=True, stop=True)
                    nc.scalar.activation(
                        out=e_all[qt][:, :, kt, :], in_=s12,
                        func=AF.Exp, scale=0.125, bias=nbias)

                def load_xns(st):
                    s0 = st * ST
                    lst = []
                    for sp in range(4):
                        xn = xn_pool.tile([128, DIN], F32R, tag="xn")
                        nc.sync.dma_start(
                            out=xn, in_=X_t.ap()[s0 + sp * 128:
                                                 s0 + (sp + 1) * 128, :]
                            .bitcast(F32R))
                        lst.append(xn)
                    return lst

                # prefetch X for the first two seq tiles ahead of the weights
                xns_bufs = [load_xns(0), load_xns(1)]

                # weights -> bf16 [128 part = din sub-chunk, NDC, TD]
                w_sb = {}
                for name, t in (("wq", Wq_t), ("wk", Wk_t), ("wv", Wv_t)):
                    stg = wst.tile([128, NDC, TD], F32, tag="wstg")
                    nc.sync.dma_start(
                        out=stg,
                        in_=t.ap().rearrange("(c p) n -> p c n", p=128))
                    w = pers.tile([128, NDC, TD], BF16, tag=name)
                    nc.scalar.copy(out=w, in_=stg)
                    w_sb[name] = w

                for st in range(NST):
                    s0 = st * ST
                    xns = xns_bufs.pop(0)
                    if st + 2 < NST:
                        xns_bufs.append(load_xns(st + 2))
                    # transpose to X^T tile [128 (din chunk), NDC, ST] bf16
                    xt = xt_pool.tile([128, NDC, ST], BF16, tag="xt")
                    for dc in range(NDC):
                        tp = tp_ps.tile([128, ST], F32R, tag="tp")
                        for sp in range(4):
                            nc.tensor.matmul(
                                tp[:, sp * 128:(sp + 1) * 128],
                                xns[sp][:, dc * 128:(dc + 1) * 128],
                                ident_r, is_transpose=True,
                                start=(sp == 0), stop=(sp == 3))
                        if st == 0:
                            nc.scalar.copy(out=xt[:, dc, :],
                                           in_=tp[:, :].bitcast(F32))
                        else:
                            nc.vector.tensor_copy(
                                xt[:, dc, :], tp[:, :].bitcast(F32))

                    # K^T slice [TD, ST]
                    kps = pj_ps.tile([128, ST], F32, tag="pj")
                    for dc in range(NDC):
                        nc.tensor.matmul(
                            kps, w_sb["wk"][:, dc, :], xt[:, dc, :],
                            start=(dc == 0), stop=(dc == NDC - 1))
                    if scores_fp8:
                        nc.vector.tensor_copy(k8n[:, s0:s0 + ST], kps)
                        for b in range(2):
                            for j in range(2):
                                nc.sync.dma_start(
                                    out=k8d[32 * b:32 * b + 32, j,
                                            s0:s0 + ST],
                                    in_=k8n[64 * b + 32 * j:
                                            64 * b + 32 * j + 32,
                                            s0:s0 + ST])
                    else:
                        nc.vector.tensor_copy(kTb[:, s0:s0 + ST], kps)

                    # Q^T slice (first NQ rows only)
                    if st < NQ // ST:
                        qps = pj_ps.tile([128, ST], F32, tag="pj")
                        for dc in range(NDC):
                            nc.tensor.matmul(
                                qps, w_sb["wq"][:, dc, :], xt[:, dc, :],
                                start=(dc == 0), stop=(dc == NDC - 1))
                        if scores_fp8:
                            nc.vector.tensor_copy(q8n[:, s0:s0 + ST], qps)
                            for b in range(2):
                                for j in range(2):
                                    nc.sync.dma_start(
                                        out=q8d[32 * b:32 * b + 32, j,
                                                s0:s0 + ST],
                                        in_=q8n[64 * b + 32 * j:
                                                64 * b + 32 * j + 32,
                                                s0:s0 + ST])
                        else:
                            nc.vector.tensor_copy(qTb[:, s0:s0 + ST], qps)

                    # V natural [s, TD] for this seq tile, interleaved with
                    # this st's qt0 scores and last st's qt1 scores
                    for sp in range(4):
                        if st == 0:
                            if sp == 0:
                                emit_kt(0, 0)
                            elif sp == 2:
                                emit_kt(0, 1)
                        elif sp < 2:
                            emit_kt(0, 4 * st + sp)
                        else:
                            emit_kt(1, 4 * (st - 1) + (sp - 2))
                        vps = pj_ps.tile([128, ST], F32, tag="pj")
                        for dc in range(NDC):
                            nc.tensor.matmul(
                                vps[:, 0:TD],
                                xt[:, dc, sp * 128:(sp + 1) * 128],
                                w_sb["wv"][:, dc, :],
                                start=(dc == 0), stop=(dc == NDC - 1))
                        nc.vector.tensor_copy(
                            v_sb[:, st * 4 + sp, :], vps[:, 0:TD])
                    if st == 0:
                        emit_kt(0, 2)
                        emit_kt(0, 3)
                    else:
                        emit_kt(0, 4 * st + 2)
                        emit_kt(1, 4 * (st - 1) + 2)
                        emit_kt(0, 4 * st + 3)
                        emit_kt(1, 4 * (st - 1) + 3)

                # tail: last qt1 scores interleaved with qt0 PV
                import concourse.mybir as _mb

                def pv_mms(o1, o2, e, k0, k1):
                    for kt in range(k0, k1):
                        first, last = (kt == 0), (kt == NKT - 1)
                        vv = v_sb[:, kt, :]
                        nc.tensor.matmul(o1, vv, e[:, 0, kt, :],
                                         start=first, stop=last)
                        nc.tensor.matmul(o2, vv, e[:, 1, kt, :],
                                         start=first, stop=last)

                def rowsums(e):
                    rinv = {}
                    for br in range(2):
                        for qc in range(4):
                            rs = tp_ps.tile([128, ST], F32R, tag="tp")
                            rsf = rs[:, :].bitcast(F32)
                            for kt in range(NKT):
                                nc.tensor.matmul(
                                    rsf[:, 0:1],
                                    e[:, br, kt, qc * 128:(qc + 1) * 128],
                                    ones8,
                                    start=(kt == 0), stop=(kt == NKT - 1))
                            ri = usb.tile([128, 1], F32, tag=f"ri{br}{qc}")
                            nc.vector.reciprocal(ri, rsf[:, 0:1])
                            rinv[(br, qc)] = ri
                    return rinv

                def epilogue(qt, o1, o2, rinv):
                    q0 = qt * QT
                    u1 = usb.tile([128, QT], F32, tag="u1")
                    u2 = usb.tile([128, QT], F32, tag="u2")
                    nc.vector.tensor_copy(u1, o1)
                    nc.vector.tensor_copy(u2, o2)
                    for qc in range(4):
                        c0 = qc * 128
                        rc2 = osb.tile([128, 1], F32, tag="rc2")
                        nc.vector.tensor_mul(rc2, rinv[(1, qc)], lam_sb)
                        uts = s_ps.tile([128, 2, QT], F32, tag="s")
                        nc.tensor.matmul(
                            uts[:, 0, 0:TD], u1[:, c0:c0 + 128], ident32,
                            is_transpose=True, start=True, stop=True)
                        nc.tensor.matmul(
                            uts[:, 1, 0:TD], u2[:, c0:c0 + 128], ident32,
                            is_transpose=True, start=True, stop=True)
                        tmp = osb.tile([128, TD], F32, tag="tmp")
                        nc.vector.tensor_scalar_mul(
                            tmp, uts[:, 1, 0:TD], rc2)
                        ob = osb.tile([128, TD], F32, tag="ob")
                        nc.vector.scalar_tensor_tensor(
                            out=ob, in0=uts[:, 0, 0:TD],
                            scalar=rinv[(0, qc)], in1=tmp,
                            op0=_mb.AluOpType.mult,
                            op1=_mb.AluOpType.subtract)
                        nc.sync.dma_start(
                            out=out_t.ap()[q0 + c0:q0 + c0 + 128, :], in_=ob)

                o1_0 = pj_ps.tile([128, ST], F32, tag="pj")
                o2_0 = pj_ps.tile([128, ST], F32, tag="pj")
                for t in range(4):
                    emit_kt(1, 4 * (NST - 1) + t)
                    pv_mms(o1_0, o2_0, e_all[0], 8 * t, 8 * (t + 1))
                rinv0 = rowsums(e_all[0])
                epilogue(0, o1_0, o2_0, rinv0)

                o1_1 = pj_ps.tile([128, ST], F32, tag="pj")
                o2_1 = pj_ps.tile([128, ST], F32, tag="pj")
                pv_mms(o1_1, o2_1, e_all[1], 0, NKT)
                rinv1 = rowsums(e_all[1])
                epilogue(1, o1_1, o2_1, rinv1)

    nc.compile()
    return nc


_NC_CACHE = None


def kernel(X, W_q, W_k, W_v, lam):
    global _NC_CACHE
    from concourse.bass_utils import run_bass_kernel_spmd

    X = np.asarray(X, dtype=np.float32)
    W_q = np.asarray(W_q, dtype=np.float32)
    W_k = np.asarray(W_k, dtype=np.float32)
    W_v = np.asarray(W_v, dtype=np.float32)
    lam_arr = np.asarray(lam, dtype=np.float32).reshape(1, 1)

    if _NC_CACHE is None:
        _NC_CACHE = build_nc()
    nc = _NC_CACHE

    in_maps = []
    for c in range(8):
        b, qc = divmod(c, 4)
        qs = qc * NQ
        Xc = np.ascontiguousarray(
            np.concatenate([X[b, qs:], X[b, :qs]], axis=0))
        in_maps.append({"X": Xc, "Wq": W_q, "Wk": W_k, "Wv": W_v,
                        "lam": lam_arr})

    res = run_bass_kernel_spmd(nc, in_maps, core_ids=list(range(8)))

    out = np.empty((B, S, TD), dtype=np.float32)
    for c in range(8):
        b, qc = divmod(c, 4)
        qs = qc * NQ
        out[b, qs:qs + NQ] = res.results[c]["out"]
    return out


# revision 43
# speedup vs baseline: 1.0747x; 1.0308x over previous
"""DiffAttn kernel for 8 Trainium2 NeuronCores.

Problem: out = softmax(Q1 K1^T / sqrt(d)) V - lam * softmax(Q2 K2^T / sqrt(d)) V
with Q = X W_q, K = X W_k, V = X W_v;  X [2, 4096, 1024], W [1024, 128], d = 64.

Sharding: 8 cores = (batch b, query-chunk qc) with b = core // 4, qc = core % 4.
Each core receives its batch's X rolled so that its 1024 query rows come first
(attention is permutation-invariant over keys).  Each core computes the full
K/V projection for its batch, Q for its query chunk, and two-branch flash
attention without max-subtraction (scores are ~N(0,1); exp(s*0.125 - 2.35)
is safe in bf16), normalizing at the end: out = U1/r1 - lam*U2/r2.

Pipeline (one TileContext):
  scope 1 (per seq tile st): DMA X rows -> PE transpose -> K^T/V/Q
  projections (bf16, V computed in natural layout directly); interleave
  attention score matmuls (bf16) + exp on the Activation engine, retaining
  exp tiles (bf16) in SBUF for all 64 (kt) x 2 (qt) tiles.
  scope 2: PV matmuls + row-sums via stationary-E 1-column matmuls (each
  accumulator in its own 2KB psum bank), then the normalize epilogue.
"""

import sys

if '/opt/trn_rl_repo' not in sys.path:
    sys.path.insert(0, '/opt/trn_rl_repo')

import numpy as np

B, S, DIN, D = 2, 4096, 1024, 64
TD = 2 * D            # 128: both branches' head dims
NQ = S // 4           # 1024 query rows per core
ST = 512              # seq tile
NST = S // ST         # 8
QT = 512              # query tile
NQT = NQ // QT        # 2
KT = 128              # key tile
NKT = S // KT         # 32
NDC = DIN // 128      # 8 contraction chunks

SCORES_FP8 = False     # fp8 DoubleRow score matmuls (else bf16)


def build_nc(scores_fp8=SCORES_FP8):
    import concourse.bacc as bacc
    import concourse.mybir as mybir
    from concourse.tile import TileContext
    from concourse.masks import make_identity

    F32 = mybir.dt.float32
    F32R = mybir.dt.float32r
    BF16 = mybir.dt.bfloat16
    F8 = mybir.dt.float8e4
    AF = mybir.ActivationFunctionType
    DR = mybir.MatmulPerfMode.DoubleRow

    nc = bacc.Bacc("TRN2", target_bir_lowering=False)
    X_t = nc.dram_tensor("X", [S, DIN], F32, kind="ExternalInput")
    Wq_t = nc.dram_tensor("Wq", [DIN, TD], F32, kind="ExternalInput")
    Wk_t = nc.dram_tensor("Wk", [DIN, TD], F32, kind="ExternalInput")
    Wv_t = nc.dram_tensor("Wv", [DIN, TD], F32, kind="ExternalInput")
    lam_t = nc.dram_tensor("lam", [1, 1], F32, kind="ExternalInput")
    out_t = nc.dram_tensor("out", [NQ, TD], F32, kind="ExternalOutput")

    with TileContext(nc) as tc:
        with tc.tile_pool(name="consts", bufs=1) as consts, \
             tc.tile_pool(name="pers", bufs=1) as pers, \
             tc.tile_pool(name="wst", bufs=2) as wst:  # noqa
            ident32 = consts.tile([128, 128], F32, tag="ident32")
            make_identity(nc, ident32)
            ident_r = consts.tile([128, 128], F32R, tag="ident_r")
            nc.scalar.copy(out=ident_r, in_=ident32)
            ones8 = consts.tile([128, 1], BF16, tag="ones8")
            nc.vector.memset(ones8, 1.0)
            nbias = consts.tile([128, 1], F32, tag="nbias")
            nc.vector.memset(nbias, -2.35)
            lam_sb = consts.tile([128, 1], F32, tag="lam")
            nc.gpsimd.dma_start(
                out=lam_sb, in_=lam_t.ap().partition_broadcast(128))

            # K^T/Q^T storage
            if scores_fp8:
                k8n = pers.tile([128, S], F8, tag="k8n")
                q8n = pers.tile([128, NQ], F8, tag="q8n")
                k8d = pers.tile([64, 2, S], F8, tag="k8d")
                q8d = pers.tile([64, 2, NQ], F8, tag="q8d")
            else:
                kTb = pers.tile([128, S], BF16, tag="kTb")
                qTb = pers.tile([128, NQ], BF16, tag="qTb")
            v_sb = pers.tile([128, NKT, TD], BF16, tag="v")
            # retained exp tiles: [128, kt, branch, q]
            e_all = []
            for qt in range(NQT):
                e_qt = pers.tile([128, 2, NKT, QT], BF16, tag=f"e{qt}")
                e_all.append(e_qt)

            # ---------- scope 1: projections + scores/exp ----------
            with tc.tile_pool(name="xn", bufs=8) as xn_pool, \
                 tc.tile_pool(name="xt", bufs=2) as xt_pool, \
                 tc.tile_pool(name="tp", bufs=2, space="PSUM") as tp_ps, \
                 tc.tile_pool(name="pj", bufs=2, space="PSUM") as pj_ps, \
                 tc.tile_pool(name="s", bufs=2, space="PSUM") as s_ps, \
                 tc.tile_pool(name="usb", bufs=1) as usb, \
                 tc.tile_pool(name="osb", bufs=2) as osb:

                def emit_kt(qt, kt):
                    """scores + exp for key tile kt against q tile qt."""
                    q0 = qt * QT
                    k0 = kt * KT
                    s12 = s_ps.tile([128, 2, QT], F32, tag="s")
                    if scores_fp8:
                        for br in range(2):
                            nc.tensor.matmul(
                                s12[:, br, :],
                                k8d[32 * br:32 * br + 32, :, k0:k0 + KT],
                                q8d[32 * br:32 * br + 32, :, q0:q0 + QT],
                                start=True, stop=True, perf_mode=DR)
                    else:
                        for br in range(2):
                            nc.tensor.matmul(
                                s12[:, br, :],
                                kTb[64 * br:64 * br + 64, k0:k0 + KT],
                                qTb[64 * br:64 * br + 64, q0:q0 + QT],
                                start=True, stop=True)
                    nc.scalar.activation(
                        out=e_all[qt][:, :, kt, :], in_=s12,
                        func=AF.Exp, scale=0.125, bias=nbias)

                def load_xns(st):
                    s0 = st * ST
                    lst = []
                    for sp in range(4):
                        xn = xn_pool.tile([128, DIN], F32R, tag="xn")
                        nc.sync.dma_start(
                            out=xn, in_=X_t.ap()[s0 + sp * 128:
                                                 s0 + (sp + 1) * 128, :]
                            .bitcast(F32R))
                        lst.append(xn)
                    return lst

                # prefetch X for the first two seq tiles ahead of the weights
                xns_bufs = [load_xns(0), load_xns(1)]

                # weights -> bf16 [128 part = din sub-chunk, NDC, TD]
                w_sb = {}
                for name, t in (("wq", Wq_t), ("wk", Wk_t), ("wv", Wv_t)):
                    stg = xn_pool.tile([128, NDC, TD], F32, tag="xn")
                    nc.sync.dma_start(
                        out=stg,
                        in_=t.ap().rearrange("(c p) n -> p c n", p=128))
                    w = pers.tile([128, NDC, TD], BF16, tag=name)
                    nc.scalar.copy(out=w, in_=stg)
                    w_sb[name] = w

                for st in range(NST):
                    s0 = st * ST
                    xns = xns_bufs.pop(0)
                    if st + 2 < NST:
                        xns_bufs.append(load_xns(st + 2))
                    # transpose to X^T tile [128 (din chunk), NDC, ST] bf16
                    xt = xt_pool.tile([128, NDC, ST], BF16, tag="xt")
                    for dc in range(NDC):
                        tp = tp_ps.tile([128, ST], F32R, tag="tp")
                        for sp in range(4):
                            nc.tensor.matmul(
                                tp[:, sp * 128:(sp + 1) * 128],
                                xns[sp][:, dc * 128:(dc + 1) * 128],
                                ident_r, is_transpose=True,
                                start=(sp == 0), stop=(sp == 3))
                        if st == 0:
                            nc.scalar.copy(out=xt[:, dc, :],
                                           in_=tp[:, :].bitcast(F32))
                        else:
                            nc.vector.tensor_copy(
                                xt[:, dc, :], tp[:, :].bitcast(F32))

                    # K^T slice [TD, ST]
                    kps = pj_ps.tile([128, ST], F32, tag="pj")
                    for dc in range(NDC):
                        nc.tensor.matmul(
                            kps, w_sb["wk"][:, dc, :], xt[:, dc, :],
                            start=(dc == 0), stop=(dc == NDC - 1))
                    if scores_fp8:
                        nc.vector.tensor_copy(k8n[:, s0:s0 + ST], kps)
                        for b in range(2):
                            for j in range(2):
                                nc.sync.dma_start(
                                    out=k8d[32 * b:32 * b + 32, j,
                                            s0:s0 + ST],
                                    in_=k8n[64 * b + 32 * j:
                                            64 * b + 32 * j + 32,
                                            s0:s0 + ST])
                    else:
                        nc.vector.tensor_copy(kTb[:, s0:s0 + ST], kps)

                    # Q^T slice (first NQ rows only)
                    if st < NQ // ST:
                        qps = pj_ps.tile([128, ST], F32, tag="pj")
                        for dc in range(NDC):
                            nc.tensor.matmul(
                                qps, w_sb["wq"][:, dc, :], xt[:, dc, :],
                                start=(dc == 0), stop=(dc == NDC - 1))
                        if scores_fp8:
                            nc.vector.tensor_copy(q8n[:, s0:s0 + ST], qps)
                            for b in range(2):
                                for j in range(2):
                                    nc.sync.dma_start(
                                        out=q8d[32 * b:32 * b + 32, j,
                                                s0:s0 + ST],
                                        in_=q8n[64 * b + 32 * j:
                                                64 * b + 32 * j + 32,
                                                s0:s0 + ST])
                        else:
                            nc.vector.tensor_copy(qTb[:, s0:s0 + ST], qps)

                    # V natural [s, TD] for this seq tile, interleaved with
                    # this st's qt0 scores and last st's qt1 scores
                    for sp in range(4):
                        if st == 0:
                            if sp == 0:
                                emit_kt(0, 0)
                            elif sp == 2:
                                emit_kt(0, 1)
                        elif sp < 2:
                            emit_kt(0, 4 * st + sp)
                        else:
                            emit_kt(1, 4 * (st - 1) + (sp - 2))
                        vps = pj_ps.tile([128, ST], F32, tag="pj")
                        for dc in range(NDC):
                            nc.tensor.matmul(
                                vps[:, 0:TD],
                                xt[:, dc, sp * 128:(sp + 1) * 128],
                                w_sb["wv"][:, dc, :],
                                start=(dc == 0), stop=(dc == NDC - 1))
                        nc.vector.tensor_copy(
                            v_sb[:, st * 4 + sp, :], vps[:, 0:TD])
                    if st == 0:
                        emit_kt(0, 2)
                        emit_kt(0, 3)
                    else:
                        emit_kt(0, 4 * st + 2)
                        emit_kt(1, 4 * (st - 1) + 2)
                        emit_kt(0, 4 * st + 3)
                        emit_kt(1, 4 * (st - 1) + 3)

                # tail: last qt1 scores interleaved with qt0 PV
                import concourse.mybir as _mb

                def pv_mms(o1, o2, e, k0, k1):
                    for kt in range(k0, k1):
                        first, last = (kt == 0), (kt == NKT - 1)
                        vv = v_sb[:, kt, :]
                        nc.tensor.matmul(o1, vv, e[:, 0, kt, :],
                                         start=first, stop=last)
                        nc.tensor.matmul(o2, vv, e[:, 1, kt, :],
                                         start=first, stop=last)

                def rowsums(e):
                    rinv = {}
                    for br in range(2):
                        for qc in range(4):
                            rs = tp_ps.tile([128, ST], F32R, tag="tp")
                            rsf = rs[:, :].bitcast(F32)
                            for kt in range(NKT):
                                nc.tensor.matmul(
                                    rsf[:, 0:1],
                                    e[:, br, kt, qc * 128:(qc + 1) * 128],
                                    ones8,
                                    start=(kt == 0), stop=(kt == NKT - 1))
                            ri = usb.tile([128, 1], F32, tag=f"ri{br}{qc}")
                            nc.vector.reciprocal(ri, rsf[:, 0:1])
                            rinv[(br, qc)] = ri
                    return rinv

                def epilogue(qt, o1, o2, rinv):
                    q0 = qt * QT
                    u1 = usb.tile([128, QT], F32, tag="u1")
                    u2 = usb.tile([128, QT], F32, tag="u2")
                    nc.vector.tensor_copy(u1, o1)
                    nc.vector.tensor_copy(u2, o2)
                    for qc in range(4):
                        c0 = qc * 128
                        rc2 = osb.tile([128, 1], F32, tag="rc2")
                        nc.vector.tensor_mul(rc2, rinv[(1, qc)], lam_sb)
                        uts = s_ps.tile([128, 2, QT], F32, tag="s")
                        nc.tensor.matmul(
                            uts[:, 0, 0:TD], u1[:, c0:c0 + 128], ident32,
                            is_transpose=True, start=True, stop=True)
                        nc.tensor.matmul(
                            uts[:, 1, 0:TD], u2[:, c0:c0 + 128], ident32,
                            is_transpose=True, start=True, stop=True)
                        tmp = osb.tile([128, TD], F32, tag="tmp")
                        nc.vector.tensor_scalar_mul(
                            tmp, uts[:, 1, 0:TD], rc2)
                        ob = osb.tile([128, TD], F32, tag="ob")
                        nc.vector.scalar_tensor_tensor(
                            out=ob, in0=uts[:, 0, 0:TD],
                            scalar=rinv[(0, qc)], in1=tmp,
                            op0=_mb.AluOpType.mult,
                            op1=_mb.AluOpType.subtract)
                        nc.sync.dma_start(
                            out=out_t.ap()[q0 + c0:q0 + c0 + 128, :], in_=ob)

                o1_0 = pj_ps.tile([128, ST], F32, tag="pj")
                o2_0 = pj_ps.tile([128, ST], F32, tag="pj")
                for t in range(4):
                    emit_kt(1, 4 * (NST - 1) + t)
                    pv_mms(o1_0, o2_0, e_all[0], 8 * t, 8 * (t + 1))
                rinv0 = rowsums(e_all[0])
                epilogue(0, o1_0, o2_0, rinv0)

                o1_1 = pj_ps.tile([128, ST], F32, tag="pj")
                o2_1 = pj_ps.tile([128, ST], F32, tag="pj")
                pv_mms(o1_1, o2_1, e_all[1], 0, NKT)
                rinv1 = rowsums(e_all[1])
                epilogue(1, o1_1, o2_1, rinv1)

    nc.compile()
    return nc


_NC_CACHE = None


def kernel(X, W_q, W_k, W_v, lam):
    global _NC_CACHE
    from concourse.bass_utils import run_bass_kernel_spmd

    X = np.asarray(X, dtype=np.float32)
    W_q = np.asarray(W_q, dtype=np.float32)
    W_k = np.asarray(W_k, dtype=np.float32)
    W_v = np.asarray(W_v, dtype=np.float32)
    lam_arr = np.asarray(lam, dtype=np.float32).reshape(1, 1)

    if _NC_CACHE is None:
        _NC_CACHE = build_nc()
    nc = _NC_CACHE

    in_maps = []
    for c in range(8):
        b, qc = divmod(c, 4)
        qs = qc * NQ
        Xc = np.ascontiguousarray(
            np.concatenate([X[b, qs:], X[b, :qs]], axis=0))
        in_maps.append({"X": Xc, "Wq": W_q, "Wk": W_k, "Wv": W_v,
                        "lam": lam_arr})

    res = run_bass_kernel_spmd(nc, in_maps, core_ids=list(range(8)))

    out = np.empty((B, S, TD), dtype=np.float32)
    for c in range(8):
        b, qc = divmod(c, 4)
        qs = qc * NQ
        out[b, qs:qs + NQ] = res.results[c]["out"]
    return out


# revision 44
# speedup vs baseline: 1.0929x; 1.0170x over previous
"""DiffAttn kernel for 8 Trainium2 NeuronCores.

Problem: out = softmax(Q1 K1^T / sqrt(d)) V - lam * softmax(Q2 K2^T / sqrt(d)) V
with Q = X W_q, K = X W_k, V = X W_v;  X [2, 4096, 1024], W [1024, 128], d = 64.

Sharding: 8 cores = (batch b, query-chunk qc) with b = core // 4, qc = core % 4.
Each core receives its batch's X rolled so that its 1024 query rows come first
(attention is permutation-invariant over keys).  Each core computes the full
K/V projection for its batch, Q for its query chunk, and two-branch flash
attention without max-subtraction (scores are ~N(0,1); exp(s*0.125 - 2.35)
is safe in bf16), normalizing at the end: out = U1/r1 - lam*U2/r2.

Pipeline (one TileContext):
  scope 1 (per seq tile st): DMA X rows -> PE transpose -> K^T/V/Q
  projections (bf16, V computed in natural layout directly); interleave
  attention score matmuls (bf16) + exp on the Activation engine, retaining
  exp tiles (bf16) in SBUF for all 64 (kt) x 2 (qt) tiles.
  scope 2: PV matmuls + row-sums via stationary-E 1-column matmuls (each
  accumulator in its own 2KB psum bank), then the normalize epilogue.
"""

import sys

if '/opt/trn_rl_repo' not in sys.path:
    sys.path.insert(0, '/opt/trn_rl_repo')

import numpy as np

B, S, DIN, D = 2, 4096, 1024, 64
TD = 2 * D            # 128: both branches' head dims
NQ = S // 4           # 1024 query rows per core
ST = 512              # seq tile
NST = S // ST         # 8
QT = 512              # query tile
NQT = NQ // QT        # 2
KT = 128              # key tile
NKT = S // KT         # 32
NDC = DIN // 128      # 8 contraction chunks

SCORES_FP8 = False     # fp8 DoubleRow score matmuls (else bf16)


def build_nc(scores_fp8=SCORES_FP8):
    import concourse.bacc as bacc
    import concourse.mybir as mybir
    from concourse.tile import TileContext
    from concourse.masks import make_identity

    F32 = mybir.dt.float32
    F32R = mybir.dt.float32r
    BF16 = mybir.dt.bfloat16
    F8 = mybir.dt.float8e4
    AF = mybir.ActivationFunctionType
    DR = mybir.MatmulPerfMode.DoubleRow

    nc = bacc.Bacc("TRN2", target_bir_lowering=False)
    X_t = nc.dram_tensor("X", [S, DIN], F32, kind="ExternalInput")
    Wq_t = nc.dram_tensor("Wq", [DIN, TD], F32, kind="ExternalInput")
    Wk_t = nc.dram_tensor("Wk", [DIN, TD], F32, kind="ExternalInput")
    Wv_t = nc.dram_tensor("Wv", [DIN, TD], F32, kind="ExternalInput")
    lam_t = nc.dram_tensor("lam", [1, 1], F32, kind="ExternalInput")
    out_t = nc.dram_tensor("out", [NQ, TD], F32, kind="ExternalOutput")

    with TileContext(nc) as tc:
        with tc.tile_pool(name="consts", bufs=1) as consts, \
             tc.tile_pool(name="pers", bufs=1) as pers, \
             tc.tile_pool(name="wst", bufs=2) as wst:  # noqa
            ident32 = consts.tile([128, 128], F32, tag="ident32")
            make_identity(nc, ident32)
            ident_r = consts.tile([128, 128], F32R, tag="ident_r")
            nc.scalar.copy(out=ident_r, in_=ident32)
            ones8 = consts.tile([128, 1], BF16, tag="ones8")
            nc.vector.memset(ones8, 1.0)
            nbias = consts.tile([128, 1], F32, tag="nbias")
            nc.vector.memset(nbias, -2.35)
            lam_sb = consts.tile([128, 1], F32, tag="lam")
            nc.gpsimd.dma_start(
                out=lam_sb, in_=lam_t.ap().partition_broadcast(128))

            # K^T/Q^T storage
            if scores_fp8:
                k8n = pers.tile([128, S], F8, tag="k8n")
                q8n = pers.tile([128, NQ], F8, tag="q8n")
                k8d = pers.tile([64, 2, S], F8, tag="k8d")
                q8d = pers.tile([64, 2, NQ], F8, tag="q8d")
            else:
                kTb = pers.tile([128, S], BF16, tag="kTb")
                qTb = pers.tile([128, NQ], BF16, tag="qTb")
            v_sb = pers.tile([128, NKT, TD], BF16, tag="v")
            # retained exp tiles: [128, kt, branch, q]
            e_all = []
            for qt in range(NQT):
                e_qt = pers.tile([128, 2, NKT, QT], BF16, tag=f"e{qt}")
                e_all.append(e_qt)

            # ---------- scope 1: projections + scores/exp ----------
            with tc.tile_pool(name="xn", bufs=8) as xn_pool, \
                 tc.tile_pool(name="xt", bufs=2) as xt_pool, \
                 tc.tile_pool(name="tp", bufs=2, space="PSUM") as tp_ps, \
                 tc.tile_pool(name="pj", bufs=2, space="PSUM") as pj_ps, \
                 tc.tile_pool(name="s", bufs=2, space="PSUM") as s_ps, \
                 tc.tile_pool(name="usb", bufs=1) as usb, \
                 tc.tile_pool(name="osb", bufs=2) as osb:

                def emit_kt(qt, kt):
                    """scores + exp for key tile kt against q tile qt."""
                    q0 = qt * QT
                    k0 = kt * KT
                    s12 = s_ps.tile([128, 2, QT], F32, tag="s")
                    if scores_fp8:
                        for br in range(2):
                            nc.tensor.matmul(
                                s12[:, br, :],
                                k8d[32 * br:32 * br + 32, :, k0:k0 + KT],
                                q8d[32 * br:32 * br + 32, :, q0:q0 + QT],
                                start=True, stop=True, perf_mode=DR)
                    else:
                        for br in range(2):
                            nc.tensor.matmul(
                                s12[:, br, :],
                                kTb[64 * br:64 * br + 64, k0:k0 + KT],
                                qTb[64 * br:64 * br + 64, q0:q0 + QT],
                                start=True, stop=True)
                    nc.scalar.activation(
                        out=e_all[qt][:, :, kt, :], in_=s12,
                        func=AF.Exp, scale=0.125, bias=nbias)

                def load_xns(st):
                    s0 = st * ST
                    lst = []
                    for sp in range(4):
                        xn = xn_pool.tile([128, DIN], F32R, tag="xn")
                        nc.sync.dma_start(
                            out=xn, in_=X_t.ap()[s0 + sp * 128:
                                                 s0 + (sp + 1) * 128, :]
                            .bitcast(F32R))
                        lst.append(xn)
                    return lst

                # prefetch X tile 0, then weights, then X tile 1
                xns_bufs = [load_xns(0)]

                # weights -> bf16 [128 part = din sub-chunk, NDC, TD]
                w_sb = {}
                for name, t in (("wq", Wq_t), ("wk", Wk_t), ("wv", Wv_t)):
                    stg = xn_pool.tile([128, NDC, TD], F32, tag="xn")
                    nc.sync.dma_start(
                        out=stg,
                        in_=t.ap().rearrange("(c p) n -> p c n", p=128))
                    w = pers.tile([128, NDC, TD], BF16, tag=name)
                    nc.scalar.copy(out=w, in_=stg)
                    w_sb[name] = w
                xns_bufs.append(load_xns(1))

                for st in range(NST):
                    s0 = st * ST
                    xns = xns_bufs.pop(0)
                    if st + 2 < NST:
                        xns_bufs.append(load_xns(st + 2))
                    # transpose to X^T tile [128 (din chunk), NDC, ST] bf16
                    xt = xt_pool.tile([128, NDC, ST], BF16, tag="xt")
                    for dc in range(NDC):
                        tp = tp_ps.tile([128, ST], F32R, tag="tp")
                        for sp in range(4):
                            nc.tensor.matmul(
                                tp[:, sp * 128:(sp + 1) * 128],
                                xns[sp][:, dc * 128:(dc + 1) * 128],
                                ident_r, is_transpose=True,
                                start=(sp == 0), stop=(sp == 3))
                        if st == 0:
                            nc.scalar.copy(out=xt[:, dc, :],
                                           in_=tp[:, :].bitcast(F32))
                        else:
                            nc.vector.tensor_copy(
                                xt[:, dc, :], tp[:, :].bitcast(F32))

                    # K^T slice [TD, ST]
                    kps = pj_ps.tile([128, ST], F32, tag="pj")
                    for dc in range(NDC):
                        nc.tensor.matmul(
                            kps, w_sb["wk"][:, dc, :], xt[:, dc, :],
                            start=(dc == 0), stop=(dc == NDC - 1))
                    if scores_fp8:
                        nc.vector.tensor_copy(k8n[:, s0:s0 + ST], kps)
                        for b in range(2):
                            for j in range(2):
                                nc.sync.dma_start(
                                    out=k8d[32 * b:32 * b + 32, j,
                                            s0:s0 + ST],
                                    in_=k8n[64 * b + 32 * j:
                                            64 * b + 32 * j + 32,
                                            s0:s0 + ST])
                    else:
                        nc.vector.tensor_copy(kTb[:, s0:s0 + ST], kps)

                    # Q^T slice (first NQ rows only)
                    if st < NQ // ST:
                        qps = pj_ps.tile([128, ST], F32, tag="pj")
                        for dc in range(NDC):
                            nc.tensor.matmul(
                                qps, w_sb["wq"][:, dc, :], xt[:, dc, :],
                                start=(dc == 0), stop=(dc == NDC - 1))
                        if scores_fp8:
                            nc.vector.tensor_copy(q8n[:, s0:s0 + ST], qps)
                            for b in range(2):
                                for j in range(2):
                                    nc.sync.dma_start(
                                        out=q8d[32 * b:32 * b + 32, j,
                                                s0:s0 + ST],
                                        in_=q8n[64 * b + 32 * j:
                                                64 * b + 32 * j + 32,
                                                s0:s0 + ST])
                        else:
                            nc.vector.tensor_copy(qTb[:, s0:s0 + ST], qps)

                    # V natural [s, TD] for this seq tile, interleaved with
                    # this st's qt0 scores and last st's qt1 scores
                    for sp in range(4):
                        if st == 0:
                            if sp == 0:
                                emit_kt(0, 0)
                            elif sp == 2:
                                emit_kt(0, 1)
                        elif sp < 2:
                            emit_kt(0, 4 * st + sp)
                        else:
                            emit_kt(1, 4 * (st - 1) + (sp - 2))
                        vps = pj_ps.tile([128, ST], F32, tag="pj")
                        for dc in range(NDC):
                            nc.tensor.matmul(
                                vps[:, 0:TD],
                                xt[:, dc, sp * 128:(sp + 1) * 128],
                                w_sb["wv"][:, dc, :],
                                start=(dc == 0), stop=(dc == NDC - 1))
                        nc.vector.tensor_copy(
                            v_sb[:, st * 4 + sp, :], vps[:, 0:TD])
                    if st == 0:
                        emit_kt(0, 2)
                        emit_kt(0, 3)
                    else:
                        emit_kt(0, 4 * st + 2)
                        emit_kt(1, 4 * (st - 1) + 2)
                        emit_kt(0, 4 * st + 3)
                        emit_kt(1, 4 * (st - 1) + 3)

                # tail: last qt1 scores interleaved with qt0 PV
                import concourse.mybir as _mb

                def pv_mms(o1, o2, e, k0, k1):
                    for kt in range(k0, k1):
                        first, last = (kt == 0), (kt == NKT - 1)
                        vv = v_sb[:, kt, :]
                        nc.tensor.matmul(o1, vv, e[:, 0, kt, :],
                                         start=first, stop=last)
                        nc.tensor.matmul(o2, vv, e[:, 1, kt, :],
                                         start=first, stop=last)

                def rowsums(e):
                    rinv = {}
                    for br in range(2):
                        for qc in range(4):
                            rs = tp_ps.tile([128, ST], F32R, tag="tp")
                            rsf = rs[:, :].bitcast(F32)
                            for kt in range(NKT):
                                nc.tensor.matmul(
                                    rsf[:, 0:1],
                                    e[:, br, kt, qc * 128:(qc + 1) * 128],
                                    ones8,
                                    start=(kt == 0), stop=(kt == NKT - 1))
                            ri = usb.tile([128, 1], F32, tag=f"ri{br}{qc}")
                            nc.vector.reciprocal(ri, rsf[:, 0:1])
                            rinv[(br, qc)] = ri
                    return rinv

                def epilogue(qt, o1, o2, rinv):
                    q0 = qt * QT
                    u1 = usb.tile([128, QT], F32, tag="u1")
                    u2 = usb.tile([128, QT], F32, tag="u2")
                    nc.vector.tensor_copy(u1, o1)
                    nc.vector.tensor_copy(u2, o2)
                    for qc in range(4):
                        c0 = qc * 128
                        rc2 = osb.tile([128, 1], F32, tag="rc2")
                        nc.vector.tensor_mul(rc2, rinv[(1, qc)], lam_sb)
                        uts = s_ps.tile([128, 2, QT], F32, tag="s")
                        nc.tensor.matmul(
                            uts[:, 0, 0:TD], u1[:, c0:c0 + 128], ident32,
                            is_transpose=True, start=True, stop=True)
                        nc.tensor.matmul(
                            uts[:, 1, 0:TD], u2[:, c0:c0 + 128], ident32,
                            is_transpose=True, start=True, stop=True)
                        tmp = osb.tile([128, TD], F32, tag="tmp")
                        nc.vector.tensor_scalar_mul(
                            tmp, uts[:, 1, 0:TD], rc2)
                        ob = osb.tile([128, TD], F32, tag="ob")
                        nc.vector.scalar_tensor_tensor(
                            out=ob, in0=uts[:, 0, 0:TD],
                            scalar=rinv[(0, qc)], in1=tmp,
                            op0=_mb.AluOpType.mult,
                            op1=_mb.AluOpType.subtract)
                        nc.sync.dma_start(
                            out=out_t.ap()[q0 + c0:q0 + c0 + 128, :], in_=ob)

                o1_0 = pj_ps.tile([128, ST], F32, tag="pj")
                o2_0 = pj_ps.tile([128, ST], F32, tag="pj")
                for t in range(4):
                    emit_kt(1, 4 * (NST - 1) + t)
                    pv_mms(o1_0, o2_0, e_all[0], 8 * t, 8 * (t + 1))
                rinv0 = rowsums(e_all[0])
                epilogue(0, o1_0, o2_0, rinv0)

                o1_1 = pj_ps.tile([128, ST], F32, tag="pj")
                o2_1 = pj_ps.tile([128, ST], F32, tag="pj")
                pv_mms(o1_1, o2_1, e_all[1], 0, NKT)
                rinv1 = rowsums(e_all[1])
                epilogue(1, o1_1, o2_1, rinv1)

    nc.compile()
    return nc


_NC_CACHE = None


def kernel(X, W_q, W_k, W_v, lam):
    global _NC_CACHE
    from concourse.bass_utils import run_bass_kernel_spmd

    X = np.asarray(X, dtype=np.float32)
    W_q = np.asarray(W_q, dtype=np.float32)
    W_k = np.asarray(W_k, dtype=np.float32)
    W_v = np.asarray(W_v, dtype=np.float32)
    lam_arr = np.asarray(lam, dtype=np.float32).reshape(1, 1)

    if _NC_CACHE is None:
        _NC_CACHE = build_nc()
    nc = _NC_CACHE

    in_maps = []
    for c in range(8):
        b, qc = divmod(c, 4)
        qs = qc * NQ
        Xc = np.ascontiguousarray(
            np.concatenate([X[b, qs:], X[b, :qs]], axis=0))
        in_maps.append({"X": Xc, "Wq": W_q, "Wk": W_k, "Wv": W_v,
                        "lam": lam_arr})

    res = run_bass_kernel_spmd(nc, in_maps, core_ids=list(range(8)))

    out = np.empty((B, S, TD), dtype=np.float32)
    for c in range(8):
        b, qc = divmod(c, 4)
        qs = qc * NQ
        out[b, qs:qs + NQ] = res.results[c]["out"]
    return out
